# revision 18
# baseline (speedup 1.0000x reference)
"""Trainium2 Bass kernel for nn_CrossEntropyLoss_22419729285187.

Computes  -sum_{matched, non-BG true rows} dot(y_true[i,1:], y_pred[rank_i]) / count
sharded over 8 NeuronCores.

Strategy (per sharding hint): the host performs the cheap key join
(encode + searchsorted + cumsum) and compacts to the contributing
(matched AND non-background) row pairs — the r-th matched true row
pairs positionally with y_pred_features[r], so both sides are plain
host gathers.  The pairs are cast to fp16 (~3e-4 relative
perturbation, far under the 2e-2 gate) and row-sharded across the 8
cores.  Per core the two shards are interleaved into ONE stream
([tile][partition][yt-line | yp-line]) so each tile is a single DMA
with 8KB contiguous per-partition lines — fewer descriptors and
completion-semaphore lanes, so tiles arrive in strict FIFO order right
behind the bytes.  The DVE runs one fused multiply-reduce
(scalar_tensor_tensor) per tile, accumulating per-tile partial sums in
fp32; a descending mini-tail keeps the post-stream DVE work ~0.3us.
The non-BG count k and the final -num/k division are host-side scalar
work.
"""

import os
import sys

for _p in ("/opt/trn_rl_repo", "/root/.axon_site/_ro/trn_rl_repo"):
    if os.path.isdir(_p) and _p not in sys.path:
        sys.path.append(_p)

import numpy as np

N_CORES = 8

PARTS = 128
G = 64  # rows per partition per tile (main segment)

_compiled = {}
_last_results = None


def _encode(idx):
    idx = idx.astype(np.int64)
    return ((idx[:, 0] * 1024 + idx[:, 1]) * 1024 + idx[:, 2]) * 1024 + idx[:, 3]


GT = 8  # rows per partition per tail mini-tile


def _plan_segments(rows):
    """Tile plan for one core: list of g (rows-per-partition) per tile.

    Uniform big tiles (g=G) plus g=GT mini-tiles covering the remainder
    (host zero-pads rows up to the plan) — every tile's matmul chunks
    are then uniform and bank-aligned, and the post-stream work is a
    few ~0.2us mini-tile ops plus a 256-element flush.
    """
    units = -(-rows // PARTS)  # 128-row units
    nbig = units // G
    rem = units - nbig * G
    return [G] * nbig + [GT] * (-(-rem // GT) if rem else 0)


def _build_program(gs, c_pred):
    """Build + schedule the SPMD Tile program for one core shard.

    gs: rows-per-partition for each tile. The single input stream is
    laid out [tile][partition][g*c yt | g*c yp] fp16, contiguous.
    """
    from concourse import bacc
    import concourse.mybir as mybir
    from concourse.tile import TileContext

    f16 = mybir.dt.float16
    f32 = mybir.dt.float32
    total = sum(2 * g * c_pred * PARTS for g in gs)
    n_tiles = len(gs)
    CH = 512                      # psum bank width (f32)
    nbig = sum(1 for g in gs if g == G)
    ntiny = n_tiles - nbig

    nc = bacc.Bacc("TRN2", target_bir_lowering=False, debug=False,
                   num_devices=N_CORES)
    x_d = nc.dram_tensor("x", [total], f16, kind="ExternalInput")
    out_d = nc.dram_tensor("partials", [1, 4], f32, kind="ExternalOutput")

    W = G * c_pred                # big-tile scr width (2048)
    WT = GT * c_pred              # mini-tile scr width (256)
    n_ch = W // CH

    with TileContext(nc) as tc:
        with tc.tile_pool(name="cst", bufs=1) as cst, \
             tc.psum_pool(name="ps", bufs=1) as psp:
            ones = cst.tile([PARTS, 1], f16)
            nc.vector.memset(ones[:], 1.0)
            res = cst.tile([1, 4], f32)
            nc.vector.memset(res[:, 3:4], 0.0)
            # psum banks 0..3 accumulate the big tiles' column sums
            # (chunk j -> bank j); the mini-tiles share one 256-col
            # region of bank 4.
            psum = psp.tile([1, (n_ch + 1) * CH], f32)
            # Per tile: DVE multiplies (tensor_tensor hits the 2x 16-bit
            # mode); the PE reduces via ones^T @ scr into psum (fp32
            # accumulation across tiles). Both run far under the stream
            # cadence, so neither instruction chain lags the DMAs.
            with tc.tile_pool(name="io", bufs=n_tiles) as pool, \
                 tc.tile_pool(name="scrp", bufs=n_tiles) as scrp:
                off = 0
                for ti, g in enumerate(gs):
                    w = 2 * g * c_pred
                    view = x_d.ap()[off:off + w * PARTS].rearrange(
                        "(p w) -> p w", p=PARTS)
                    off += w * PARTS
                    xt = pool.tile([PARTS, w], f16, tag="x")
                    nc.sync.dma_start(out=xt[:], in_=view)
                    wid = g * c_pred
                    scr = scrp.tile([PARTS, wid], f16, tag="scr")
                    nc.vector.tensor_tensor(
                        out=scr[:], in0=xt[:, :wid],
                        in1=xt[:, wid:], op=mybir.AluOpType.mult)
                    if g == G:
                        for j in range(n_ch):
                            nc.tensor.matmul(
                                psum[:, j * CH:(j + 1) * CH],
                                ones[:], scr[:, j * CH:(j + 1) * CH],
                                start=(ti == 0), stop=(ti == nbig - 1))
                    else:
                        nc.tensor.matmul(
                            psum[:, n_ch * CH:n_ch * CH + WT],
                            ones[:], scr[:],
                            start=(ti == nbig), stop=(ti == n_tiles - 1))
                # Flush A (big banks) runs as soon as the big-tile
                # matmuls retire, overlapping the mini-tile tail; its
                # result ships immediately so the HBM write receipt
                # also overlaps. Flush B is a ~0.3us mini reduce.
                half = (n_ch // 2) * CH
                dmp = cst.tile([1, max(half, CH)], f32)
                nc.scalar.activation(
                    out=dmp[:, :half], in_=psum[:, :half],
                    func=mybir.ActivationFunctionType.Copy,
                    accum_out=res[:, 0:1])
                nc.vector.tensor_reduce(
                    out=res[:, 1:2], in_=psum[:, half:n_ch * CH],
                    axis=mybir.AxisListType.X, op=mybir.AluOpType.add)
                nc.scalar.dma_start(out=out_d[:, 0:2], in_=res[:, 0:2])
                if ntiny > 0:
                    nc.vector.tensor_reduce(
                        out=res[:, 2:3],
                        in_=psum[:, n_ch * CH:n_ch * CH + WT],
                        axis=mybir.AxisListType.X, op=mybir.AluOpType.add)
                else:
                    nc.vector.memset(res[:, 2:3], 0.0)
                nc.scalar.dma_start(out=out_d[:, 2:4], in_=res[:, 2:4])
    nc.compile()
    return nc


def kernel(y_true_features, y_true_indices, y_pred_features, y_pred_indices):
    global _last_results
    from concourse.bass_utils import run_bass_kernel_spmd

    yt = np.asarray(y_true_features, dtype=np.float32)
    yp = np.asarray(y_pred_features, dtype=np.float32)
    n, c1 = yt.shape
    m, c = yp.shape

    # ---- host-side key join (cheap integer work) ----
    kt = _encode(np.asarray(y_true_indices))
    kp = _encode(np.asarray(y_pred_indices))
    kps = np.sort(kp)
    pos = np.clip(np.searchsorted(kps, kt), 0, m - 1)
    matched = kps[pos] == kt
    # Only matched, non-background true rows contribute. The r-th
    # matched true row (row order) pairs with y_pred_features[r]
    # positionally (rank = cumsum(matched)-1 is sequential over matched
    # rows), so compacting to the contributing pairs is two host
    # gathers; k is their count.
    midx = np.flatnonzero(matched)
    keep = np.flatnonzero(yt[midx, 0] != 1.0)   # positions within matched
    k = keep.size
    yt_cmp = yt[midx[keep], 1:].astype(np.float16)   # [k, c]
    yp_cmp = yp[keep].astype(np.float16)             # [k, c]

    # ---- shard the k contributing pairs across cores ----
    rows = -(-k // N_CORES)
    gs = _plan_segments(rows)
    r_pad = PARTS * sum(gs)

    key = (tuple(gs), c)
    if key not in _compiled:
        _compiled[key] = _build_program(gs, c)
    nc = _compiled[key]

    total = 2 * r_pad * c
    in_maps = []
    for i in range(N_CORES):
        lo, hi = i * rows, min((i + 1) * rows, k)
        nr = max(hi - lo, 0)
        a = np.zeros((r_pad, c), dtype=np.float16)
        a[:nr] = yt_cmp[lo:hi]
        b = np.zeros((r_pad, c), dtype=np.float16)
        b[:nr] = yp_cmp[lo:hi]
        # interleave per tile: [tile][partition][g*c of a | g*c of b]
        x = np.empty(total, dtype=np.float16)
        off = 0
        r0 = 0
        for g in gs:
            trows = PARTS * g
            w = 2 * g * c
            blk = x[off:off + trows * 2 * c].reshape(PARTS, w)
            blk[:, :g * c] = a[r0:r0 + trows].reshape(PARTS, g * c)
            blk[:, g * c:] = b[r0:r0 + trows].reshape(PARTS, g * c)
            off += trows * 2 * c
            r0 += trows
        in_maps.append({"x": x})

    res = run_bass_kernel_spmd(nc, in_maps, list(range(N_CORES)))
    _last_results = res

    num = 0.0
    for i in range(N_CORES):
        num += float(np.asarray(res.results[i]["partials"],
                                dtype=np.float64).sum())
    return np.float32(-num / k)


# revision 21
# speedup vs baseline: 1.1137x; 1.1137x over previous
"""Trainium2 Bass kernel for nn_CrossEntropyLoss_22419729285187.

Computes  -sum_{matched, non-BG true rows} dot(y_true[i,1:], y_pred[rank_i]) / count
sharded over 8 NeuronCores.

Strategy (per sharding hint): the host performs the cheap key join
(encode + searchsorted + cumsum) and compacts to the contributing
(matched AND non-background) row pairs — the r-th matched true row
pairs positionally with y_pred_features[r], so both sides are plain
host gathers.  The pairs are cast to fp16 (~3e-4 relative
perturbation, far under the 2e-2 gate) and row-sharded across the 8
cores.  Per core the two shards are interleaved into ONE stream
([tile][partition][yt-line | yp-line]) so each tile is a single DMA
with 8KB contiguous per-partition lines — fewer descriptors and
completion-semaphore lanes, so tiles arrive in strict FIFO order right
behind the bytes.  The DVE runs one fused multiply-reduce
(scalar_tensor_tensor) per tile, accumulating per-tile partial sums in
fp32; a descending mini-tail keeps the post-stream DVE work ~0.3us.
The non-BG count k and the final -num/k division are host-side scalar
work.
"""

import os
import sys

for _p in ("/opt/trn_rl_repo", "/root/.axon_site/_ro/trn_rl_repo"):
    if os.path.isdir(_p) and _p not in sys.path:
        sys.path.append(_p)

import numpy as np

N_CORES = 8

PARTS = 128
G = 64  # rows per partition per tile (main segment)

_compiled = {}
_last_results = None


def _encode(idx):
    idx = idx.astype(np.int64)
    return ((idx[:, 0] * 1024 + idx[:, 1]) * 1024 + idx[:, 2]) * 1024 + idx[:, 3]


def _plan_segments(rows):
    """Tile plan for one core: list of g (rows-per-partition) per tile.

    Uniform big tiles (g=G) reduced via the PE, plus a short descending
    tail (32/16/8 units, host zero-pads rows up to the plan) reduced
    via scalar_tensor_tensor right as each lands — the post-stream
    critical path is a ~0.3us STT, not a multi-us flush.
    """
    units = -(-rows // PARTS)  # 128-row units
    nbig = units // G
    rem = units - nbig * G
    if rem > 56:               # tail caps at 32+16+8; round up to a big tile
        return [G] * (nbig + 1)
    gs = [G] * nbig
    for t in (32, 16, 8):
        if rem <= 0:
            break
        if rem >= t or t == 8:
            gs.append(t)
            rem -= t
    return gs


def _build_program(gs, c_pred):
    """Build + schedule the SPMD Tile program for one core shard.

    gs: rows-per-partition for each tile. The single input stream is
    laid out [tile][partition][g*c yt | g*c yp] fp16, contiguous.
    """
    from concourse import bacc
    import concourse.mybir as mybir
    from concourse.tile import TileContext

    f16 = mybir.dt.float16
    f32 = mybir.dt.float32
    total = sum(2 * g * c_pred * PARTS for g in gs)
    n_tiles = len(gs)
    CH = 512                      # psum bank width (f32)
    nbig = sum(1 for g in gs if g == G)
    ntiny = n_tiles - nbig

    nc = bacc.Bacc("TRN2", target_bir_lowering=False, debug=False,
                   num_devices=N_CORES)
    x_d = nc.dram_tensor("x", [total], f16, kind="ExternalInput")
    outa_d = nc.dram_tensor("pa", [1, 2], f32, kind="ExternalOutput")
    ntail = max(ntiny, 1)
    outb_d = nc.dram_tensor("pb", [PARTS, ntail], f32, kind="ExternalOutput")

    W = G * c_pred                # big-tile scr width (2048)
    n_ch = W // CH

    with TileContext(nc) as tc:
        with tc.tile_pool(name="cst", bufs=1) as cst, \
             tc.psum_pool(name="ps", bufs=1) as psp:
            ones = cst.tile([PARTS, 1], f16)
            nc.vector.memset(ones[:], 1.0)
            res = cst.tile([1, 2], f32)
            red = cst.tile([PARTS, ntail], f32)
            if ntiny == 0:
                nc.vector.memset(red[:], 0.0)
            # psum banks 0..3 accumulate the big tiles' column sums
            # (chunk j -> bank j).
            psum = psp.tile([1, n_ch * CH], f32)
            # Big tiles: DVE multiplies (tensor_tensor hits the 2x
            # 16-bit mode), the PE reduces via ones^T @ scr into psum.
            # Tail tiles: one fused scalar_tensor_tensor each (short,
            # runs right as the tile lands). Every chain stays under
            # the stream cadence.
            with tc.tile_pool(name="io", bufs=n_tiles) as pool, \
                 tc.tile_pool(name="scrp", bufs=nbig) as scrp, \
                 tc.tile_pool(name="dmpp", bufs=1) as dmpp:
                dmp = dmpp.tile([PARTS, 32 * c_pred], f16)
                off = 0
                for ti, g in enumerate(gs):
                    w = 2 * g * c_pred
                    view = x_d.ap()[off:off + w * PARTS].rearrange(
                        "(p w) -> p w", p=PARTS)
                    off += w * PARTS
                    xt = pool.tile([PARTS, w], f16, tag="x")
                    nc.sync.dma_start(out=xt[:], in_=view)
                    wid = g * c_pred
                    if g == G:
                        scr = scrp.tile([PARTS, wid], f16, tag="scr")
                        nc.vector.tensor_tensor(
                            out=scr[:], in0=xt[:, :wid],
                            in1=xt[:, wid:], op=mybir.AluOpType.mult)
                        for j in range(n_ch):
                            nc.tensor.matmul(
                                psum[:, j * CH:(j + 1) * CH],
                                ones[:], scr[:, j * CH:(j + 1) * CH],
                                start=(ti == 0), stop=(ti == nbig - 1))
                    else:
                        tj = ti - nbig
                        nc.vector.scalar_tensor_tensor(
                            out=dmp[:, :wid], in0=xt[:, :wid], scalar=1.0,
                            in1=xt[:, wid:],
                            op0=mybir.AluOpType.mult,
                            op1=mybir.AluOpType.mult,
                            accum_out=red[:, tj:tj + 1])
                # Flush (big banks) runs as soon as the big-tile
                # matmuls retire, overlapping the tail; its result
                # ships immediately so the HBM write receipt overlaps
                # the tail too. ACT takes 3 banks, DVE 1.
                asz = 3 * CH
                dmpf = cst.tile([1, asz], f32)
                if nbig > 0:
                    nc.scalar.activation(
                        out=dmpf[:], in_=psum[:, :asz],
                        func=mybir.ActivationFunctionType.Copy,
                        accum_out=res[:, 0:1])
                    nc.vector.tensor_reduce(
                        out=res[:, 1:2], in_=psum[:, asz:],
                        axis=mybir.AxisListType.X, op=mybir.AluOpType.add)
                else:
                    nc.vector.memset(res[:], 0.0)
                nc.scalar.dma_start(out=outa_d[:], in_=res[:])
                nc.sync.dma_start(out=outb_d[:], in_=red[:])
    nc.compile()
    return nc


def kernel(y_true_features, y_true_indices, y_pred_features, y_pred_indices):
    global _last_results
    from concourse.bass_utils import run_bass_kernel_spmd

    yt = np.asarray(y_true_features, dtype=np.float32)
    yp = np.asarray(y_pred_features, dtype=np.float32)
    n, c1 = yt.shape
    m, c = yp.shape

    # ---- host-side key join (cheap integer work) ----
    kt = _encode(np.asarray(y_true_indices))
    kp = _encode(np.asarray(y_pred_indices))
    kps = np.sort(kp)
    pos = np.clip(np.searchsorted(kps, kt), 0, m - 1)
    matched = kps[pos] == kt
    # Only matched, non-background true rows contribute. The r-th
    # matched true row (row order) pairs with y_pred_features[r]
    # positionally (rank = cumsum(matched)-1 is sequential over matched
    # rows), so compacting to the contributing pairs is two host
    # gathers; k is their count.
    midx = np.flatnonzero(matched)
    keep = np.flatnonzero(yt[midx, 0] != 1.0)   # positions within matched
    k = keep.size
    yt_cmp = yt[midx[keep], 1:].astype(np.float16)   # [k, c]
    yp_cmp = yp[keep].astype(np.float16)             # [k, c]

    # ---- shard the k contributing pairs across cores ----
    rows = -(-k // N_CORES)
    gs = _plan_segments(rows)
    r_pad = PARTS * sum(gs)

    key = (tuple(gs), c)
    if key not in _compiled:
        _compiled[key] = _build_program(gs, c)
    nc = _compiled[key]

    total = 2 * r_pad * c
    in_maps = []
    for i in range(N_CORES):
        lo, hi = i * rows, min((i + 1) * rows, k)
        nr = max(hi - lo, 0)
        a = np.zeros((r_pad, c), dtype=np.float16)
        a[:nr] = yt_cmp[lo:hi]
        b = np.zeros((r_pad, c), dtype=np.float16)
        b[:nr] = yp_cmp[lo:hi]
        # interleave per tile: [tile][partition][g*c of a | g*c of b]
        x = np.empty(total, dtype=np.float16)
        off = 0
        r0 = 0
        for g in gs:
            trows = PARTS * g
            w = 2 * g * c
            blk = x[off:off + trows * 2 * c].reshape(PARTS, w)
            blk[:, :g * c] = a[r0:r0 + trows].reshape(PARTS, g * c)
            blk[:, g * c:] = b[r0:r0 + trows].reshape(PARTS, g * c)
            off += trows * 2 * c
            r0 += trows
        in_maps.append({"x": x})

    res = run_bass_kernel_spmd(nc, in_maps, list(range(N_CORES)))
    _last_results = res

    num = 0.0
    for i in range(N_CORES):
        num += float(np.asarray(res.results[i]["pa"], dtype=np.float64).sum())
        num += float(np.asarray(res.results[i]["pb"], dtype=np.float64).sum())
    return np.float32(-num / k)


# revision 26
# speedup vs baseline: 1.1176x; 1.0035x over previous
"""Trainium2 Bass kernel for nn_CrossEntropyLoss_22419729285187.

Computes  -sum_{matched, non-BG true rows} dot(y_true[i,1:], y_pred[rank_i]) / count
sharded over 8 NeuronCores.

Strategy (per sharding hint): the host performs the cheap key join
(encode + searchsorted + cumsum) and compacts to the contributing
(matched AND non-background) row pairs — the r-th matched true row
pairs positionally with y_pred_features[r], so both sides are plain
host gathers.  The pairs are cast to fp16 (~3e-4 relative
perturbation, far under the 2e-2 gate) and row-sharded across the 8
cores.  Per core the two shards are interleaved into ONE stream
([tile][partition][yt-line | yp-line]) so each tile is a single DMA
with 8KB contiguous per-partition lines — fewer descriptors and
completion-semaphore lanes, so tiles arrive in strict FIFO order right
behind the bytes.  The DVE runs one fused multiply-reduce
(scalar_tensor_tensor) per tile, accumulating per-tile partial sums in
fp32; a descending mini-tail keeps the post-stream DVE work ~0.3us.
The non-BG count k and the final -num/k division are host-side scalar
work.
"""

import os
import sys

for _p in ("/opt/trn_rl_repo", "/root/.axon_site/_ro/trn_rl_repo"):
    if os.path.isdir(_p) and _p not in sys.path:
        sys.path.append(_p)

import numpy as np

N_CORES = 8

PARTS = 128
G = 64  # rows per partition per tile (main segment)

_compiled = {}
_last_results = None


def _encode(idx):
    idx = idx.astype(np.int64)
    return ((idx[:, 0] * 1024 + idx[:, 1]) * 1024 + idx[:, 2]) * 1024 + idx[:, 3]


def _plan_segments(rows):
    """Tile plan for one core: list of g (rows-per-partition) per tile.

    Uniform big tiles (g=G) reduced via the PE, plus a short descending
    tail (32/16/8 units, host zero-pads rows up to the plan) reduced
    via scalar_tensor_tensor right as each lands — the post-stream
    critical path is a ~0.3us STT, not a multi-us flush.
    """
    units = -(-rows // PARTS)  # 128-row units
    nbig = units // G
    rem = units - nbig * G
    if rem > 56:               # tail caps at 32+16+8; round up to a big tile
        return [G] * (nbig + 1)
    gs = [G] * nbig
    for t in (32, 16, 8):
        if rem <= 0:
            break
        if rem >= t or t == 8:
            gs.append(t)
            rem -= t
    return gs


def _build_program(gs, c_pred):
    """Build + schedule the SPMD Tile program for one core shard.

    gs: rows-per-partition for each tile. The single input stream is
    laid out [tile][partition][g*c yt | g*c yp] fp16, contiguous.
    """
    from concourse import bacc
    import concourse.mybir as mybir
    from concourse.tile import TileContext

    f16 = mybir.dt.float16
    f32 = mybir.dt.float32
    total = sum(2 * g * c_pred * PARTS for g in gs)
    n_tiles = len(gs)
    CH = 512                      # psum bank width (f32)
    nbig = sum(1 for g in gs if g == G)
    ntiny = n_tiles - nbig

    nc = bacc.Bacc("TRN2", target_bir_lowering=False, debug=False,
                   num_devices=N_CORES)
    x_d = nc.dram_tensor("x", [total], f16, kind="ExternalInput")
    outa_d = nc.dram_tensor("pa", [1, 2], f32, kind="ExternalOutput")
    ntail = max(ntiny, 1)
    outb_d = nc.dram_tensor("pb", [PARTS, ntail], f32, kind="ExternalOutput")

    W = G * c_pred                # big-tile scr width (2048)
    n_ch = W // CH

    with TileContext(nc) as tc:
        with tc.tile_pool(name="cst", bufs=1) as cst, \
             tc.psum_pool(name="ps", bufs=1) as psp:
            ones = cst.tile([PARTS, 1], f16)
            nc.vector.memset(ones[:], 1.0)
            res = cst.tile([1, 2], f32)
            nc.vector.memset(res[:, 1:2], 0.0)
            red = cst.tile([PARTS, ntail], f32)
            if ntiny == 0:
                nc.vector.memset(red[:], 0.0)
            # One psum bank accumulates every big-tile chunk's column
            # sums (the PE serializes its own read-modify-write), so
            # the final flush is a single 512-element pass.
            psum = psp.tile([1, CH], f32)
            # Big tiles: DVE multiplies (tensor_tensor hits the 2x
            # 16-bit mode), the PE reduces via ones^T @ scr into psum.
            # Tail tiles: one fused scalar_tensor_tensor each (short,
            # runs right as the tile lands). Every chain stays under
            # the stream cadence.
            with tc.tile_pool(name="io", bufs=n_tiles) as pool, \
                 tc.tile_pool(name="scrp", bufs=nbig) as scrp, \
                 tc.tile_pool(name="dmpp", bufs=1) as dmpp:
                dmp = dmpp.tile([PARTS, 32 * c_pred], f16)
                off = 0
                for ti, g in enumerate(gs):
                    w = 2 * g * c_pred
                    view = x_d.ap()[off:off + w * PARTS].rearrange(
                        "(p w) -> p w", p=PARTS)
                    off += w * PARTS
                    xt = pool.tile([PARTS, w], f16, tag="x")
                    nc.sync.dma_start(out=xt[:], in_=view)
                    wid = g * c_pred
                    if g == G:
                        scr = scrp.tile([PARTS, wid], f16, tag="scr")
                        nc.vector.tensor_tensor(
                            out=scr[:], in0=xt[:, :wid],
                            in1=xt[:, wid:], op=mybir.AluOpType.mult)
                        for j in range(n_ch):
                            nc.tensor.matmul(
                                psum[:],
                                ones[:], scr[:, j * CH:(j + 1) * CH],
                                start=(ti == 0 and j == 0),
                                stop=(ti == nbig - 1 and j == n_ch - 1))
                    else:
                        tj = ti - nbig
                        nc.vector.scalar_tensor_tensor(
                            out=dmp[:, :wid], in0=xt[:, :wid], scalar=1.0,
                            in1=xt[:, wid:],
                            op0=mybir.AluOpType.mult,
                            op1=mybir.AluOpType.mult,
                            accum_out=red[:, tj:tj + 1])
                # Flush runs on ACT (DVE stays free for the tail STTs)
                # as soon as the big-tile matmuls retire; its result
                # ships immediately so the HBM write receipt overlaps
                # the tail.
                dmpf = cst.tile([1, CH], f32)
                if nbig > 0:
                    nc.scalar.activation(
                        out=dmpf[:], in_=psum[:],
                        func=mybir.ActivationFunctionType.Copy,
                        accum_out=res[:, 0:1])
                else:
                    nc.vector.memset(res[:, 0:1], 0.0)
                nc.scalar.dma_start(out=outa_d[:], in_=res[:])
                nc.sync.dma_start(out=outb_d[:], in_=red[:])
    nc.compile()
    return nc


def kernel(y_true_features, y_true_indices, y_pred_features, y_pred_indices):
    global _last_results
    from concourse.bass_utils import run_bass_kernel_spmd

    yt = np.asarray(y_true_features, dtype=np.float32)
    yp = np.asarray(y_pred_features, dtype=np.float32)
    n, c1 = yt.shape
    m, c = yp.shape

    # ---- host-side key join (cheap integer work) ----
    kt = _encode(np.asarray(y_true_indices))
    kp = _encode(np.asarray(y_pred_indices))
    kps = np.sort(kp)
    pos = np.clip(np.searchsorted(kps, kt), 0, m - 1)
    matched = kps[pos] == kt
    # Only matched, non-background true rows contribute. The r-th
    # matched true row (row order) pairs with y_pred_features[r]
    # positionally (rank = cumsum(matched)-1 is sequential over matched
    # rows), so compacting to the contributing pairs is two host
    # gathers; k is their count.
    midx = np.flatnonzero(matched)
    keep = np.flatnonzero(yt[midx, 0] != 1.0)   # positions within matched
    k = keep.size
    yt_cmp = yt[midx[keep], 1:].astype(np.float16)   # [k, c]
    yp_cmp = yp[keep].astype(np.float16)             # [k, c]

    # ---- shard the k contributing pairs across cores ----
    rows = -(-k // N_CORES)
    gs = _plan_segments(rows)
    r_pad = PARTS * sum(gs)

    key = (tuple(gs), c)
    if key not in _compiled:
        _compiled[key] = _build_program(gs, c)
    nc = _compiled[key]

    total = 2 * r_pad * c
    in_maps = []
    for i in range(N_CORES):
        lo, hi = i * rows, min((i + 1) * rows, k)
        nr = max(hi - lo, 0)
        a = np.zeros((r_pad, c), dtype=np.float16)
        a[:nr] = yt_cmp[lo:hi]
        b = np.zeros((r_pad, c), dtype=np.float16)
        b[:nr] = yp_cmp[lo:hi]
        # interleave per tile: [tile][partition][g*c of a | g*c of b]
        x = np.empty(total, dtype=np.float16)
        off = 0
        r0 = 0
        for g in gs:
            trows = PARTS * g
            w = 2 * g * c
            blk = x[off:off + trows * 2 * c].reshape(PARTS, w)
            blk[:, :g * c] = a[r0:r0 + trows].reshape(PARTS, g * c)
            blk[:, g * c:] = b[r0:r0 + trows].reshape(PARTS, g * c)
            off += trows * 2 * c
            r0 += trows
        in_maps.append({"x": x})

    res = run_bass_kernel_spmd(nc, in_maps, list(range(N_CORES)))
    _last_results = res

    num = 0.0
    for i in range(N_CORES):
        num += float(np.asarray(res.results[i]["pa"], dtype=np.float64).sum())
        num += float(np.asarray(res.results[i]["pb"], dtype=np.float64).sum())
    return np.float32(-num / k)


# revision 29
# speedup vs baseline: 1.1323x; 1.0132x over previous
"""Trainium2 Bass kernel for nn_CrossEntropyLoss_22419729285187.

Computes  -sum_{matched, non-BG true rows} dot(y_true[i,1:], y_pred[rank_i]) / count
sharded over 8 NeuronCores.

Strategy (per sharding hint): the host performs the cheap key join
(encode + searchsorted + cumsum) and compacts to the contributing
(matched AND non-background) row pairs — the r-th matched true row
pairs positionally with y_pred_features[r], so both sides are plain
host gathers.  The pairs are cast to fp16 (~6e-4 relative
perturbation, far under the 2e-2 gate) and row-sharded across the 8
cores.  Per core the two shards are interleaved into ONE stream
([tile][partition][yt-line | yp-line]) so each tile is a single DMA
with large contiguous per-partition lines; ~10 DMAs stay within the 8
completion-semaphore lanes, so tiles arrive in strict FIFO order right
behind the bytes (~23us for 8.3MB, at the per-core HBM roofline).
Big tiles: the DVE multiplies (tensor_tensor in the 2x 16-bit mode)
and the idle PE reduces via ones^T @ scr, accumulating column sums in
one PSUM bank across tiles; the ACT engine flushes that bank once,
overlapped with the tail.  A descending 32/16/8-unit tail runs fused
scalar_tensor_tensor right as each tile lands, so the post-last-byte
critical path is one short STT plus two tiny result DMAs.  The non-BG
count k and the final -num/k division are host-side scalar work.
"""

import os
import sys

for _p in ("/opt/trn_rl_repo", "/root/.axon_site/_ro/trn_rl_repo"):
    if os.path.isdir(_p) and _p not in sys.path:
        sys.path.append(_p)

import numpy as np

N_CORES = 8

PARTS = 128
G = 64  # rows per partition per tile (main segment)

_compiled = {}
_last_results = None


def _encode(idx):
    idx = idx.astype(np.int64)
    return ((idx[:, 0] * 1024 + idx[:, 1]) * 1024 + idx[:, 2]) * 1024 + idx[:, 3]


def _plan_segments(rows):
    """Tile plan for one core: list of g (rows-per-partition) per tile.

    Uniform big tiles (g=G) reduced via the PE, plus a short descending
    tail (32/16/8 units, host zero-pads rows up to the plan) reduced
    via scalar_tensor_tensor right as each lands — the post-stream
    critical path is a ~0.3us STT, not a multi-us flush.
    """
    units = -(-rows // PARTS)  # 128-row units
    nbig = units // G
    rem = units - nbig * G
    if rem > 56:               # tail caps at 32+16+8; round up to a big tile
        return [G] * (nbig + 1)
    gs = [G] * nbig
    for t in (32, 16, 8):
        if rem <= 0:
            break
        if rem >= t or t == 8:
            gs.append(t)
            rem -= t
    return gs


def _build_program(gs, c_pred):
    """Build + schedule the SPMD Tile program for one core shard.

    gs: rows-per-partition for each tile. The single input stream is
    laid out [tile][partition][g*c yt | g*c yp] fp16, contiguous.
    """
    from concourse import bacc
    import concourse.mybir as mybir
    from concourse.tile import TileContext

    f16 = mybir.dt.float16
    f32 = mybir.dt.float32
    total = sum(2 * g * c_pred * PARTS for g in gs)
    n_tiles = len(gs)
    CH = 512                      # psum bank width (f32)
    nbig = sum(1 for g in gs if g == G)
    ntiny = n_tiles - nbig

    nc = bacc.Bacc("TRN2", target_bir_lowering=False, debug=False,
                   num_devices=N_CORES)
    x_d = nc.dram_tensor("x", [total], f16, kind="ExternalInput")
    outa_d = nc.dram_tensor("pa", [1, 2], f32, kind="ExternalOutput")
    ntail = max(ntiny, 1)
    outb_d = nc.dram_tensor("pb", [PARTS, ntail], f32, kind="ExternalOutput")

    W = G * c_pred                # big-tile scr width (2048)
    n_ch = W // CH

    with TileContext(nc) as tc:
        with tc.tile_pool(name="cst", bufs=1) as cst, \
             tc.psum_pool(name="ps", bufs=1) as psp:
            ones = cst.tile([PARTS, 1], f16)
            nc.vector.memset(ones[:], 1.0)
            res = cst.tile([1, 2], f32)
            nc.vector.memset(res[:, 1:2], 0.0)
            red = cst.tile([PARTS, ntail], f32)
            if ntiny == 0:
                nc.vector.memset(red[:], 0.0)
            # One psum bank accumulates every big-tile chunk's column
            # sums (the PE serializes its own read-modify-write), so
            # the final flush is a single 512-element pass.
            psum = psp.tile([1, CH], f32)
            # Big tiles: DVE multiplies (tensor_tensor hits the 2x
            # 16-bit mode), the PE reduces via ones^T @ scr into psum.
            # Tail tiles: one fused scalar_tensor_tensor each (short,
            # runs right as the tile lands). Every chain stays under
            # the stream cadence.
            with tc.tile_pool(name="io", bufs=n_tiles) as pool, \
                 tc.tile_pool(name="scrp", bufs=nbig) as scrp, \
                 tc.tile_pool(name="dmpp", bufs=1) as dmpp:
                dmp = dmpp.tile([PARTS, 32 * c_pred], f16)
                off = 0
                for ti, g in enumerate(gs):
                    w = 2 * g * c_pred
                    view = x_d.ap()[off:off + w * PARTS].rearrange(
                        "(p w) -> p w", p=PARTS)
                    off += w * PARTS
                    xt = pool.tile([PARTS, w], f16, tag="x")
                    nc.sync.dma_start(out=xt[:], in_=view)
                    wid = g * c_pred
                    if g == G:
                        scr = scrp.tile([PARTS, wid], f16, tag="scr")
                        nc.vector.tensor_tensor(
                            out=scr[:], in0=xt[:, :wid],
                            in1=xt[:, wid:], op=mybir.AluOpType.mult)
                        for j in range(n_ch):
                            nc.tensor.matmul(
                                psum[:],
                                ones[:], scr[:, j * CH:(j + 1) * CH],
                                start=(ti == 0 and j == 0),
                                stop=(ti == nbig - 1 and j == n_ch - 1))
                    else:
                        tj = ti - nbig
                        nc.vector.scalar_tensor_tensor(
                            out=dmp[:, :wid], in0=xt[:, :wid], scalar=1.0,
                            in1=xt[:, wid:],
                            op0=mybir.AluOpType.mult,
                            op1=mybir.AluOpType.mult,
                            accum_out=red[:, tj:tj + 1])
                # Flush runs on ACT (DVE stays free for the tail STTs)
                # as soon as the big-tile matmuls retire; its result
                # ships immediately so the HBM write receipt overlaps
                # the tail.
                dmpf = cst.tile([1, CH], f32)
                if nbig > 0:
                    nc.scalar.activation(
                        out=dmpf[:], in_=psum[:],
                        func=mybir.ActivationFunctionType.Copy,
                        accum_out=res[:, 0:1])
                else:
                    nc.vector.memset(res[:, 0:1], 0.0)
                nc.scalar.dma_start(out=outa_d[:], in_=res[:])
                nc.sync.dma_start(out=outb_d[:], in_=red[:])
    nc.compile()
    return nc


def _ensure_axon_hooks():
    """bass_utils imports antenv.axon_hooks when BASS_TRACE is set; some
    images ship an antenv stub without it. Provide the module (and the
    ctypes NTFF hook, so tracing still works) only when it's missing."""
    try:
        import antenv.axon_hooks  # noqa: F401
        return
    except ImportError:
        pass
    import types
    try:
        import antenv
    except ImportError:
        return
    m = types.ModuleType("antenv.axon_hooks")
    m._hook = None
    def _set(h, _m=m):
        _m._hook = h
    def _get(_m=m):
        return _m._hook
    m.set_axon_ntff_profile_hook = _set
    m.get_axon_ntff_profile_hook = _get
    sys.modules["antenv.axon_hooks"] = m
    antenv.axon_hooks = m
    try:
        from trn_agent_boot.trn_boot import _ntff_profile_via_ctypes
        hook = _ntff_profile_via_ctypes("/opt/axon/libaxon_pjrt.so")
        if hook is not None:
            _set(hook)
    except Exception:
        pass


def kernel(y_true_features, y_true_indices, y_pred_features, y_pred_indices):
    global _last_results
    _ensure_axon_hooks()
    from concourse.bass_utils import run_bass_kernel_spmd

    yt = np.asarray(y_true_features, dtype=np.float32)
    yp = np.asarray(y_pred_features, dtype=np.float32)
    n, c1 = yt.shape
    m, c = yp.shape

    # ---- host-side key join (cheap integer work) ----
    kt = _encode(np.asarray(y_true_indices))
    kp = _encode(np.asarray(y_pred_indices))
    kps = np.sort(kp)
    pos = np.clip(np.searchsorted(kps, kt), 0, m - 1)
    matched = kps[pos] == kt
    # Only matched, non-background true rows contribute. The r-th
    # matched true row (row order) pairs with y_pred_features[r]
    # positionally (rank = cumsum(matched)-1 is sequential over matched
    # rows), so compacting to the contributing pairs is two host
    # gathers; k is their count.
    midx = np.flatnonzero(matched)
    keep = np.flatnonzero(yt[midx, 0] != 1.0)   # positions within matched
    k = keep.size
    yt_cmp = yt[midx[keep], 1:].astype(np.float16)   # [k, c]
    yp_cmp = yp[keep].astype(np.float16)             # [k, c]

    # ---- shard the k contributing pairs across cores ----
    rows = -(-k // N_CORES)
    gs = _plan_segments(rows)
    r_pad = PARTS * sum(gs)

    key = (tuple(gs), c)
    if key not in _compiled:
        _compiled[key] = _build_program(gs, c)
    nc = _compiled[key]

    total = 2 * r_pad * c
    in_maps = []
    for i in range(N_CORES):
        lo, hi = i * rows, min((i + 1) * rows, k)
        nr = max(hi - lo, 0)
        a = np.zeros((r_pad, c), dtype=np.float16)
        a[:nr] = yt_cmp[lo:hi]
        b = np.zeros((r_pad, c), dtype=np.float16)
        b[:nr] = yp_cmp[lo:hi]
        # interleave per tile: [tile][partition][g*c of a | g*c of b]
        x = np.empty(total, dtype=np.float16)
        off = 0
        r0 = 0
        for g in gs:
            trows = PARTS * g
            w = 2 * g * c
            blk = x[off:off + trows * 2 * c].reshape(PARTS, w)
            blk[:, :g * c] = a[r0:r0 + trows].reshape(PARTS, g * c)
            blk[:, g * c:] = b[r0:r0 + trows].reshape(PARTS, g * c)
            off += trows * 2 * c
            r0 += trows
        in_maps.append({"x": x})

    try:
        res = run_bass_kernel_spmd(nc, in_maps, list(range(N_CORES)))
    except Exception:
        # Transient NRT exec errors (wedged device state) clear on retry.
        res = run_bass_kernel_spmd(nc, in_maps, list(range(N_CORES)))
    _last_results = res

    num = 0.0
    for i in range(N_CORES):
        num += float(np.asarray(res.results[i]["pa"], dtype=np.float64).sum())
        num += float(np.asarray(res.results[i]["pb"], dtype=np.float64).sum())
    return np.float32(-num / k)
